# revision 20
# baseline (speedup 1.0000x reference)
"""BiLSTM Trainium2 kernel.

Problem: B=64, T=512, D=U=512. Two independent LSTMs (fwd on xf, bwd on xb),
outputs concatenated on the feature dim.

Sharding: direction-split x batch-split. Cores 0-3 run the forward LSTM
(16 batch rows each), cores 4-7 the backward LSTM. No collectives; the
per-core weights/inputs differ only through the input maps (same SPMD
program on all 8 cores).

Per core:
  Phase 1 (proj): xz = x @ W + b as a dense matmul over all T*B_loc rows
    (bias folded in via a K=1 ones-row matmul), written to a DRAM scratch.
  Phase 2 (recurrence): for t in range(T):
      z = h_{t-1} @ U + xz_t   (float32r matmuls, xz_t injected into the
                                same PSUM accumulation via an identity
                                stationary matmul)
      i,f,o = sigmoid(z[...]), g = tanh(z[...])   (gate columns pre-permuted
                                to [i|f|o|g] per 256-col chunk so one ACT
                                instruction covers i,f,o)
      c = f*c + i*g ; h = o*tanh(c)
      h transposed back to [U, B] layout via PE-transpose for the next
      step's stationary operand. Two 256-column chunks per step pipeline
      the ACT/DVE tail under the PE streaming of the next chunk/step.
"""

import os
import sys

sys.path.insert(0, "/opt/trn_rl_repo")

import numpy as np
import ml_dtypes
from contextlib import ExitStack

import concourse.bass as bass  # noqa: F401
import concourse.tile as tile
from concourse import bacc, mybir
from concourse.bass_utils import run_bass_kernel_spmd

B, T, D, U = 64, 512, 512, 512
G = 4 * U                      # gate width 2048
NCORE = 8
NDIR_CORES = 4                 # cores per direction
B_LOC = B // NDIR_CORES        # 16
NCHUNK = int(os.environ.get("BK_NCHUNK", "4"))  # h-column chunks per step
CH = U // NCHUNK
LAST_FIRST = int(os.environ.get("BK_LAST_FIRST", "0"))  # finish last chunk's bank first
SPLIT_SIG = int(os.environ.get("BK_SPLIT_SIG", "0"))    # separate if-sig and o-sig
TG_FIRST = int(os.environ.get("BK_TG_FIRST", "0"))      # emit tanh_g before sigmoid
GP_FC = int(os.environ.get("BK_GP_FC", "1"))            # fc on gpsimd
SIGALL = int(os.environ.get("BK_SIGALL", "1"))          # one sigmoid for all 4 gates (g pre-scaled x2)
GP_AFF = int(os.environ.get("BK_GP_AFF", "1"))          # g-affine fixup on gpsimd
PAIR = int(os.environ.get("BK_PAIR", "0"))              # single z psum tile; chains at 256-col pair granularity
DMAT = int(os.environ.get("BK_DMAT", "0"))              # h transpose via DMA xbar (bf16 hT)

F32 = mybir.dt.float32
F32R = mybir.dt.float32r
BF16 = mybir.dt.bfloat16
AF = mybir.ActivationFunctionType


def _gate_perm():
    """New gate-column order: per 256-chunk c: [i_c, f_c, o_c, g_c].

    Original Keras order along 4U: [i(0:U), f(U:2U), g(2U:3U), o(3U:4U)].
    """
    idx = []
    for c in range(NCHUNK):
        s = c * CH
        for g0 in (0, U, 3 * U, 2 * U):  # i, f, o, g
            idx.append(np.arange(g0 + s, g0 + s + CH))
    return np.concatenate(idx)


def _emit(tc, nc, xT, Wp, Up, eye, eyer, onesr, zerosr, zerosb, hs, t_steps, b_loc):
    rt = t_steps * b_loc
    n_m = rt // 128

    with ExitStack() as es:
        consts = es.enter_context(tc.tile_pool(name="consts", bufs=1))
        dramp = es.enter_context(tc.tile_pool(name="dram", bufs=1, space="DRAM"))

        xz = dramp.tile([rt, G], F32R, tag="xz")

        w_t = consts.tile([128, 4, G], F32R, tag="w")
        u_t = consts.tile([128, 4, G], F32R, tag="u")
        for k in range(4):
            nc.sync.dma_start(out=w_t[:, k, :], in_=Wp[128 * k:128 * (k + 1), :])
            nc.sync.dma_start(out=u_t[:, k, :], in_=Up[128 * k:128 * (k + 1), :])
        wb_t = consts.tile([1, G], F32R, tag="wb")
        nc.sync.dma_start(out=wb_t, in_=Wp[D:D + 1, :])
        eye_t = consts.tile([b_loc, b_loc], F32, tag="eye")
        nc.sync.dma_start(out=eye_t, in_=eye)
        eyer_t = consts.tile([b_loc, b_loc], F32R, tag="eyer")
        nc.sync.dma_start(out=eyer_t, in_=eyer)
        ones_t = consts.tile([1, 128], F32R, tag="ones")
        nc.sync.dma_start(out=ones_t, in_=onesr)

        # ---- xz = x @ W + b : emission helper --------------------------
        # The first PRE m-tiles are emitted before the recurrence loop; the
        # rest are interleaved one-per-8-steps into the loop so their matmuls
        # fill the PE bubbles while it waits on the gate chain, and xz stays
        # ~128 steps ahead of consumption.
        es2 = es.enter_context(ExitStack())
        xkp = es2.enter_context(tc.tile_pool(name="xk", bufs=3))
        pcp = es2.enter_context(tc.tile_pool(name="pc", bufs=3))
        pjps = es2.enter_context(tc.tile_pool(name="pjps", bufs=2,
                                              space="PSUM"))

        xk_tiles = {}

        def emit_proj_n(m, n):
            if n == 0:
                xk = xkp.tile([128, 4, 128], F32R, tag="xk", name=f"xk_{m}")
                nc.sync.dma_start(
                    out=xk,
                    in_=xT[:, 128 * m:128 * (m + 1)].rearrange(
                        "(k p) m -> p k m", p=128),
                )
                xk_tiles[m] = xk
            xk = xk_tiles[m]
            ps = pjps.tile([128, 512], F32, tag="pj", name=f"pj_{m}_{n}")
            for k in range(4):
                nc.tensor.matmul(
                    ps, xk[:, k, :], w_t[:, k, 512 * n:512 * (n + 1)],
                    start=(k == 0), stop=False)
            nc.tensor.matmul(
                ps, ones_t, wb_t[:, 512 * n:512 * (n + 1)],
                start=False, stop=True)
            oc = pcp.tile([128, 512], F32R, tag="oc", name=f"oc_{m}_{n}")
            nc.vector.tensor_copy(oc, ps)
            nc.sync.dma_start(
                out=xz[128 * m:128 * (m + 1), 512 * n:512 * (n + 1)],
                in_=oc)

        def emit_proj(m):
            for n in range(4):
                emit_proj_n(m, n)

        PRE = min(n_m, 16)
        for m in range(PRE):
            emit_proj(m)
        next_m = PRE * 4   # counted in quarters now

        # ---- Phase 2: recurrence ----------------------------------------
        with tc.tile_pool(name="xzp", bufs=3) as xzp, \
             tc.tile_pool(name="state", bufs=2) as state, \
             tc.tile_pool(name="gates", bufs=3) as gp, \
             tc.tile_pool(name="zps",
                          bufs=(1 if PAIR else 4 // (4 * CH // 512)),
                          space="PSUM") as zps, \
             tc.tile_pool(name="hps", bufs=2, space="PSUM") as hps:

            n_state = NCHUNK // 2 if PAIR else NCHUNK
            kslices = 4 // n_state
            hT_prev = [state.tile([128, kslices * b_loc],
                                  BF16 if DMAT else F32R,
                                  tag=f"hT{c}", name=f"hT_init{c}")
                       for c in range(n_state)]
            c_prev = [state.tile([b_loc, (4 // n_state) * CH], F32,
                                 tag=(f"cp{c}" if PAIR else f"c{c}"),
                                 name=f"c_init{c}")
                      for c in range(n_state)]
            for c in range(n_state):
                zsrc = (zerosb if DMAT else zerosr)
                nc.sync.dma_start(out=hT_prev[c],
                                  in_=zsrc[:, :kslices * b_loc])
                nc.vector.memset(c_prev[c][:], 0.0)

            for t in range(t_steps):
                xz_t = xzp.tile([b_loc, G], F32R, tag="xz_t")
                nc.sync.dma_start(out=xz_t, in_=xz[b_loc * t:b_loc * (t + 1), :])
                # chunk c covers z columns [c*4*CH, (c+1)*4*CH) and h/c
                # columns [c*CH, (c+1)*CH); KPC k-tiles per chunk.
                KPC = 4 // NCHUNK
                CW = 4 * CH                 # z-columns per chunk
                JPC = CW // 512             # 512-wide matmul slices per chunk
                if PAIR:
                    zbig = zps.tile([b_loc, G], F32, tag="z",
                                    name=f"z_{t}")
                    pss = [zbig[:, CW * c:CW * (c + 1)]
                           for c in range(NCHUNK)]
                else:
                    pss = [zps.tile([b_loc, CW], F32, tag="z",
                                    name=f"z_{t}_{c}") for c in range(NCHUNK)]

                def mm(c, j, k, start=False, stop=False):
                    n0 = CW * c + 512 * j
                    pj = pss[c][:, 512 * j:512 * (j + 1)]
                    if k == 4:
                        nc.tensor.matmul(pj, eyer_t, xz_t[:, n0:n0 + 512],
                                         start=start, stop=stop,
                                         skip_group_check=True)
                    else:
                        kpt = 4 // len(hT_prev)
                        src_c, kk = divmod(k, kpt)
                        nc.tensor.matmul(
                            pj,
                            hT_prev[src_c][:, b_loc * kk:b_loc * (kk + 1)],
                            u_t[:, k, n0:n0 + 512],
                            start=start, stop=stop, skip_group_check=True)

                # xz injects first: they have no h dependency, so they fill
                # the PE bubble at step start and each bank then completes at
                # its last k-matmul instead of waiting for a trailing inject.
                for c in range(NCHUNK):
                    for j in range(JPC):
                        mm(c, j, 4, start=True)
                # k-rounds ordered by which hT chunk they need, so the PE can
                # start as soon as the earliest chunk of hT(t-1) lands.
                for r in range(NCHUNK - 1):
                    for c in range(NCHUNK):
                        for j in range(JPC):
                            for k in range(r * KPC, (r + 1) * KPC):
                                mm(c, j, k)
                hT_new, c_new = [None] * NCHUNK, [None] * NCHUNK
                hns = [None] * NCHUNK
                corder = (list(range(NCHUNK - 1, -1, -1)) if LAST_FIRST
                          else list(range(NCHUNK)))
                if PAIR:
                    for pr in range(NCHUNK // 2):
                        c0, c1 = 2 * pr, 2 * pr + 1
                        for c in (c0, c1):
                            for j in range(JPC):
                                for k in range((NCHUNK - 1) * KPC,
                                               NCHUNK * KPC):
                                    mm(c, j, k,
                                       stop=(k == NCHUNK * KPC - 1))
                        W2 = 2 * CH
                        ps2 = zbig[:, CW * c0:CW * (c1 + 1)]
                        sig = gp.tile([b_loc, 2 * CW], F32, tag=f"sigp{pr}",
                                      name=f"sig_{t}_{pr}")
                        nc.scalar.activation(sig, ps2, AF.Sigmoid)
                        tg = gp.tile([b_loc, W2], F32, tag=f"tgp{pr}",
                                     name=f"tg_{t}_{pr}")
                        aff_eng = nc.gpsimd if GP_AFF else nc.vector
                        for ci in range(2):
                            aff_eng.tensor_scalar(
                                tg[:, CH * ci:CH * (ci + 1)],
                                sig[:, CW * ci + 3 * CH:CW * ci + 4 * CH],
                                2.0, 1.0, mybir.AluOpType.mult,
                                mybir.AluOpType.subtract)
                        fc = gp.tile([b_loc, W2], F32, tag=f"fcp{pr}",
                                     name=f"fc_{t}_{pr}")
                        fc_eng = nc.gpsimd if GP_FC else nc.vector
                        cpv = c_prev[pr]
                        for ci in range(2):
                            fc_eng.tensor_mul(
                                fc[:, CH * ci:CH * (ci + 1)],
                                sig[:, CW * ci + CH:CW * ci + 2 * CH],
                                cpv[:, CH * ci:CH * (ci + 1)])
                        ig = gp.tile([b_loc, W2], F32, tag=f"igp{pr}",
                                     name=f"ig_{t}_{pr}")
                        for ci in range(2):
                            nc.vector.tensor_mul(
                                ig[:, CH * ci:CH * (ci + 1)],
                                sig[:, CW * ci:CW * ci + CH],
                                tg[:, CH * ci:CH * (ci + 1)])
                        cn = state.tile([b_loc, W2], F32, tag=f"cp{pr}",
                                        name=f"c_{t}_{pr}")
                        nc.vector.tensor_add(cn, ig, fc)
                        tch = gp.tile([b_loc, W2], F32, tag=f"tcp{pr}",
                                      name=f"tc_{t}_{pr}")
                        nc.scalar.activation(tch, cn, AF.Tanh)
                        hn = gp.tile([b_loc, W2], F32, tag=f"hp{pr}",
                                     name=f"h_{t}_{pr}")
                        for ci in range(2):
                            nc.vector.tensor_mul(
                                hn[:, CH * ci:CH * (ci + 1)],
                                sig[:, CW * ci + 2 * CH:CW * ci + 3 * CH],
                                tch[:, CH * ci:CH * (ci + 1)])
                        nc.sync.dma_start(
                            out=hs[t, :, W2 * pr:W2 * (pr + 1)], in_=hn)
                        c_new[pr] = cn
                        hns[pr] = hn

                    if t % 2 == 0 and next_m < 4 * n_m:
                        emit_proj_n(next_m // 4, next_m % 4)
                        next_m += 1

                    for pr in range(NCHUNK // 2):
                        hn = hns[pr]
                        hT = state.tile([128, 2 * b_loc], F32R,
                                        tag=f"hT{pr}", name=f"hT_{t}_{pr}")
                        for kk in range(2):
                            pt = hps.tile([128, b_loc], F32, tag="pt",
                                          name=f"pt_{t}_{pr}_{kk}")
                            nc.tensor.transpose(
                                pt, hn[:, CH * kk:CH * (kk + 1)], eye_t)
                            nc.vector.tensor_copy(
                                hT[:, b_loc * kk:b_loc * (kk + 1)], pt)
                        hT_new[pr] = hT
                    hT_prev, c_prev = hT_new[:2], c_new[:2]
                    continue
                for c in corder:
                    for j in range(JPC):
                        for k in range((NCHUNK - 1) * KPC, NCHUNK * KPC):
                            mm(c, j, k, stop=(k == NCHUNK * KPC - 1))
                    ps = pss[c]
                    if SIGALL:
                        # g columns were pre-scaled x2 host-side;
                        # tanh(x) = 2*sigmoid(2x) - 1 makes one sigmoid
                        # instruction cover all four gates of the chunk.
                        sig = gp.tile([b_loc, 4 * CH], F32, tag=f"sig{c}",
                                      name=f"sig_{t}_{c}")
                        nc.scalar.activation(sig, ps[:, 0:4 * CH], AF.Sigmoid)
                        tg = gp.tile([b_loc, CH], F32, tag=f"tg{c}",
                                     name=f"tg_{t}_{c}")
                        aff_eng = nc.gpsimd if GP_AFF else nc.vector
                        aff_eng.tensor_scalar(
                            tg, sig[:, 3 * CH:4 * CH], 2.0, 1.0,
                            mybir.AluOpType.mult, mybir.AluOpType.subtract)
                    else:
                        sig = gp.tile([b_loc, 3 * CH], F32, tag=f"sig{c}",
                                      name=f"sig_{t}_{c}")
                        nc.scalar.activation(sig, ps[:, 0:3 * CH], AF.Sigmoid)
                        tg = gp.tile([b_loc, CH], F32, tag=f"tg{c}",
                                     name=f"tg_{t}_{c}")
                        nc.scalar.activation(tg, ps[:, 3 * CH:4 * CH], AF.Tanh)
                    fc = gp.tile([b_loc, CH], F32, tag=f"fc{c}",
                                 name=f"fc_{t}_{c}")
                    fc_eng = nc.gpsimd if GP_FC else nc.vector
                    fc_eng.tensor_mul(fc, sig[:, CH:2 * CH], c_prev[c])
                    ig = gp.tile([b_loc, CH], F32, tag=f"ig{c}",
                                 name=f"ig_{t}_{c}")
                    nc.vector.tensor_mul(ig, sig[:, 0:CH], tg)
                    cn = state.tile([b_loc, CH], F32, tag=f"c{c}", name=f"c_{t}_{c}")
                    nc.vector.tensor_add(cn, ig, fc)
                    tch = gp.tile([b_loc, CH], F32, tag=f"tc{c}",
                                  name=f"tc_{t}_{c}")
                    nc.scalar.activation(tch, cn, AF.Tanh)
                    hn = gp.tile([b_loc, CH], F32, tag=f"h{c}",
                                 name=f"h_{t}_{c}")
                    nc.vector.tensor_mul(hn, sig[:, 2 * CH:3 * CH], tch)
                    nc.sync.dma_start(
                        out=hs[t, :, CH * c:CH * (c + 1)], in_=hn)
                    hns[c] = hn
                    c_new[c] = cn

                # proj work lands here in PE program order: it fills the
                # bubble while the PE waits for the first gate chain.
                if t % 2 == 0 and next_m < 4 * n_m:
                    emit_proj_n(next_m // 4, next_m % 4)
                    next_m += 1

                for c in corder:
                    hn = hns[c]
                    hT = state.tile([128, KPC * b_loc], F32R,
                                    tag=f"hT{c}", name=f"hT_{t}_{c}")
                    for kk in range(KPC):
                        pt = hps.tile([128, b_loc], F32, tag="pt",
                                      name=f"pt_{t}_{c}_{kk}")
                        nc.tensor.transpose(
                            pt, hn[:, 128 * kk:128 * (kk + 1)], eye_t)
                        nc.vector.tensor_copy(
                            hT[:, b_loc * kk:b_loc * (kk + 1)], pt)
                    hT_new[c] = hT
                hT_prev, c_prev = hT_new, c_new


def build_program(t_steps=T, b_loc=B_LOC):
    rt = t_steps * b_loc
    nc = bacc.Bacc("TRN2", target_bir_lowering=False, debug=False,
                   num_devices=NCORE)
    xT = nc.dram_tensor("xT", [D, rt], F32R, kind="ExternalInput").ap()
    Wp = nc.dram_tensor("Wp", [D + 1, G], F32R, kind="ExternalInput").ap()
    Up = nc.dram_tensor("Up", [U, G], F32R, kind="ExternalInput").ap()
    eye = nc.dram_tensor("eye", [b_loc, b_loc], F32, kind="ExternalInput").ap()
    eyer = nc.dram_tensor("eyer", [b_loc, b_loc], F32R,
                          kind="ExternalInput").ap()
    onesr = nc.dram_tensor("onesr", [1, 128], F32R, kind="ExternalInput").ap()
    zerosr = nc.dram_tensor("zerosr", [128, 2 * b_loc], F32R,
                            kind="ExternalInput").ap()
    zerosb = nc.dram_tensor("zerosb", [128, 2 * b_loc], mybir.dt.bfloat16,
                            kind="ExternalInput").ap()
    hs = nc.dram_tensor("hs", [t_steps, b_loc, U], F32,
                        kind="ExternalOutput").ap()
    with tile.TileContext(nc) as tc:
        _emit(tc, nc, xT, Wp, Up, eye, eyer, onesr, zerosr, zerosb, hs, t_steps, b_loc)
    nc.compile()
    return nc


_CACHE = {}


def _get_program(t_steps=T, b_loc=B_LOC):
    key = (t_steps, b_loc)
    if key not in _CACHE:
        _CACHE[key] = build_program(t_steps, b_loc)
    return _CACHE[key]


def make_in_maps(xf, xb, Wf, Uf, bf, Wb, Ub, bb, t_steps=T, b_loc=B_LOC):
    perm = _gate_perm()
    eye = np.eye(b_loc, dtype=np.float32)
    packs = {}
    gscale = np.ones(G, np.float32)
    if SIGALL:
        for c in range(NCHUNK):
            gscale[(4 * c + 3) * CH:(4 * c + 4) * CH] = 2.0
    for d, (W, Urec, bias) in enumerate(((Wf, Uf, bf), (Wb, Ub, bb))):
        Wp = np.ascontiguousarray(
            np.concatenate([W, bias[None, :]], axis=0)[:, perm] * gscale)
        Upp = np.ascontiguousarray(Urec[:, perm] * gscale)
        packs[d] = (Wp, Upp)
    in_maps = []
    for core in range(NCORE):
        d, j = divmod(core, NDIR_CORES)
        x = (xf if d == 0 else xb)[b_loc * j:b_loc * (j + 1), :t_steps]
        # xT[d, t*b_loc + b] = x[b, t, d]
        xT = np.ascontiguousarray(
            x.transpose(2, 1, 0).reshape(D, t_steps * b_loc))
        Wp, Upp = packs[d]
        in_maps.append({"xT": xT, "Wp": Wp, "Up": Upp, "eye": eye,
                        "eyer": eye,
                        "onesr": np.ones((1, 128), np.float32),
                        "zerosr": np.zeros((128, 2 * b_loc), np.float32),
                        "zerosb": np.zeros((128, 2 * b_loc),
                                           ml_dtypes.bfloat16)})
    return in_maps


def kernel(xf, xb, Wf, Uf, bf, Wb, Ub, bb):
    xf = np.asarray(xf, np.float32)
    xb = np.asarray(xb, np.float32)
    Wf = np.asarray(Wf, np.float32)
    Uf = np.asarray(Uf, np.float32)
    bf = np.asarray(bf, np.float32)
    Wb = np.asarray(Wb, np.float32)
    Ub = np.asarray(Ub, np.float32)
    bb = np.asarray(bb, np.float32)

    nc = _get_program()
    in_maps = make_in_maps(xf, xb, Wf, Uf, bf, Wb, Ub, bb)
    res = run_bass_kernel_spmd(nc, in_maps, list(range(NCORE)))

    out = np.empty((B, T, 2 * U), np.float32)
    for core in range(NCORE):
        d, j = divmod(core, NDIR_CORES)
        hsv = res.results[core]["hs"]  # [T, b_loc, U]
        out[B_LOC * j:B_LOC * (j + 1), :, U * d:U * (d + 1)] = \
            hsv.transpose(1, 0, 2)
    return out
